# revision 21
# baseline (speedup 1.0000x reference)
import sys

if "/opt/trn_rl_repo" not in sys.path:
    sys.path.insert(0, "/opt/trn_rl_repo")

import numpy as np
import ml_dtypes

from concourse import bass, tile, bacc
from concourse.bass import mybir

F32 = mybir.dt.float32
F16 = mybir.dt.float16
BF16 = mybir.dt.bfloat16
I16 = mybir.dt.int16
I8 = mybir.dt.int8

N_CORES = 8
N_TOTAL = 32768
N_CORE = N_TOTAL // N_CORES  # 4096 rows per core
D = 1024
C = 64
K = 16
DEPTH = 4
M = 1024
G = 128                      # xT DRAM row granularity (floats)
STAGES = [512, 512, 1024, 2048]  # rows per stage (sum = N_CORE)
N_WARM_MM = 100                  # dummy matmuls to keep HAM warm during prologue
ALU = mybir.AluOpType
AFT = mybir.ActivationFunctionType

assert sum(STAGES) == N_CORE
_bases = np.cumsum([0] + STAGES[:-1]).tolist()
for _w, _b in zip(STAGES, _bases):
    assert _b % (_w // 2) == 0 and (_w // 2) % G == 0


def build_program(dims, repeat=1):
    """dims kept for signature compat; gather indices live in gidx input."""
    nc = bacc.Bacc()
    # xT row 16*u + g holds x[g*256:(g+1)*256, u] of this core's shard
    xT_d = nc.declare_dram_parameter("xT", [N_CORE // G * D, G], F32, isOutput=False)
    thr_d = nc.declare_dram_parameter("thrcols", [128, 15], F32, isOutput=False)
    lut_d = nc.declare_dram_parameter("lutT", [C * K, M], BF16, isOutput=False)
    ktab_d = nc.declare_dram_parameter("ktab", [128, 8], F32, isOutput=False)
    gidx_d = nc.declare_dram_parameter("gidx", [128, 32 * len(STAGES)], I16,
                                       isOutput=False)
    out_d = nc.declare_dram_parameter("out", [N_CORE, M], F16, isOutput=True)

    with tile.TileContext(nc) as tc:
        from contextlib import ExitStack
        es = ExitStack()
        pers = es.enter_context(tc.tile_pool(name="pers", bufs=1))

        def ptile(shape, dtype, name):
            return pers.tile(shape, dtype, name=name, tag=name)

        WMAX = max(STAGES)

        # ---- persistent tiles ----
        lutT = ptile([128, 8, M], BF16, "lutT_sb")       # [tau*128+64e+c, tau, m]
        thr = ptile([128, 15], F32, "thr_sb")
        ktab = ptile([128, 8], F32, "ktab_sb")
        gidx = ptile([128, 32 * len(STAGES)], I16, "gidx_sb")
        tmps = [ptile([128, WMAX // 2], F32, f"tmp{ti}_sb") for ti in range(7)]
        b0, b1, b2, sa, sb, sc, sd = tmps
        bi = ptile([128, WMAX], I16, "bi_sb")
        b0i = bi[:, :WMAX // 2]
        b1i = bi[:, WMAX // 2:]

        chpool = es.enter_context(tc.tile_pool(name="chpool", bufs=2))
        bkpool = es.enter_context(tc.tile_pool(name="bkpool", bufs=2))
        b2pool = es.enter_context(tc.tile_pool(name="b2pool", bufs=2))
        etpool = es.enter_context(tc.tile_pool(name="etpool", bufs=2))
        opool = es.enter_context(tc.tile_pool(name="opool", bufs=4))
        pspool = es.enter_context(
            tc.tile_pool(name="pspool", bufs=4, space=bass.MemorySpace.PSUM)
        )

        nc.sync.dma_start(gidx[:], gidx_d[:])
        nc.sync.dma_start(thr[:], thr_d[:])
        nc.sync.dma_start(ktab[:], ktab_d[:])
        for j in range(8):
            nc.sync.dma_start(lutT[:, j, :], lut_d[j * 128:(j + 1) * 128, :])

        # dummy matmuls keep the PE clock un-throttled through the prologue
        wps = pspool.tile([128, 512], F32, name="wps", tag="ps0")
        for _ in range(N_WARM_MM):
            nc.tensor.matmul(wps[:], lutT[:, 0, 0:128], lutT[:, 0, 0:512],
                             start=True, stop=True)

        def tcol(i):
            return thr[:, i:i + 1]

        from concourse.tile import add_dep_helper
        last_iseq = None
        stage_list = [sw for _ in range(repeat) for sw in zip(range(len(STAGES)),
                                                              STAGES, _bases)]
        for s, W, base in stage_list:
            HW_ = W // 2  # half-stage width
            # ---- gather: ch[p=(h,c), d, n'] = x[base+h*HW_+n', dims[4c+d]]
            ch = chpool.tile([128, DEPTH, HW_], F32, name="ch", tag="ch")
            b = HW_ // G
            src = xT_d[:] if b == 1 else xT_d[:].rearrange(
                "(a b) g -> a (b g)", b=b)
            g0 = s * 32
            nc.gpsimd.dma_gather(
                ch[:], src, gidx[:, g0:g0 + 32], 512, 512, HW_,
            )
            xd = [ch[:, d, :] for d in range(DEPTH)]

            def T(t):
                return t[:, :HW_]

            # ---- tree descent on [128=(h,c), HW_] ----
            i0 = nc.vector.tensor_scalar(T(b0), xd[0], tcol(0), None, ALU.is_gt)
            if last_iseq is not None:
                add_dep_helper(i0.ins, last_iseq.ins, sync=False,
                               reason="DVE order: ET compares before next descent")
            nc.vector.tensor_scalar(T(sa), T(b0), tcol(2), tcol(1), ALU.mult, ALU.add)
            nc.vector.tensor_copy(T(b0i), T(b0))
            nc.vector.tensor_tensor(T(b1), xd[1], T(sa), ALU.is_gt)

            nc.vector.tensor_scalar(T(sa), T(b1), tcol(4), tcol(3), ALU.mult, ALU.add)
            nc.vector.tensor_scalar(T(sb), T(b1), tcol(6), tcol(5), ALU.mult, ALU.add)
            nc.vector.tensor_copy(T(b1i), T(b1))
            nc.vector.copy_predicated(T(sa), T(b0i), T(sb))
            nc.vector.tensor_tensor(T(b2), xd[2], T(sa), ALU.is_gt)

            nc.vector.tensor_scalar(T(sa), T(b2), tcol(8), tcol(7), ALU.mult, ALU.add)
            nc.vector.tensor_scalar(T(sb), T(b2), tcol(10), tcol(9), ALU.mult, ALU.add)
            nc.vector.tensor_scalar(T(sc), T(b2), tcol(12), tcol(11), ALU.mult, ALU.add)
            nc.vector.tensor_scalar(T(sd), T(b2), tcol(14), tcol(13), ALU.mult, ALU.add)
            nc.vector.copy_predicated(T(sa), T(b1i), T(sb))
            nc.vector.copy_predicated(T(sc), T(b1i), T(sd))
            nc.vector.copy_predicated(T(sa), T(b0i), T(sc))
            nc.vector.tensor_tensor(T(sb), xd[3], T(sa), ALU.is_gt)  # b3 -> sb

            bk = bkpool.tile([128, HW_], BF16, name="bk", tag="bk")
            nc.vector.scalar_tensor_tensor(T(sc), T(b0), 2.0, T(b1), ALU.mult, ALU.add)
            nc.vector.scalar_tensor_tensor(T(sd), T(sc), 2.0, T(b2), ALU.mult, ALU.add)
            nc.vector.scalar_tensor_tensor(bk[:], T(sd), 2.0, T(sb), ALU.mult, ALU.add)

            # ---- duplicate bucket to both e-halves: b2k[64e+c, h*HW_+n'] ----
            b2k = b2pool.tile([128, W], BF16, name="b2k", tag="b2k")
            for e in range(2):
                for h in range(2):
                    _eng = nc.scalar if (s == 0 and h == 1) else nc.sync
                    _eng.dma_start(
                        b2k[64 * e:64 * e + 64, h * HW_:(h + 1) * HW_],
                        bk[64 * h:64 * h + 64, :],
                    )

            # ---- ET: et[p=(e,c), tau, nn] = (bucket == 2*tau + e) ----
            et = etpool.tile([128, 8, W], BF16, name="et", tag="et")
            for tau in range(8):
                last_iseq = nc.vector.tensor_scalar(
                    et[:, tau, :], b2k[:], ktab[:, tau:tau + 1], None, ALU.is_equal
                )

            # ---- matmul + output ----
            for i in range(W // 128):
                ps = [
                    pspool.tile([128, 512], F32, name=f"ps{mc}", tag=f"ps{mc}")
                    for mc in range(2)
                ]
                for tau in range(8):
                    lhsT = et[:, tau, i * 128:(i + 1) * 128]
                    for mc in range(2):
                        nc.tensor.matmul(
                            ps[mc][:], lhsT, lutT[:, tau, mc * 512:(mc + 1) * 512],
                            start=(tau == 0), stop=(tau == 7),
                        )
                osb = opool.tile([128, M], F16, name="osb", tag="osb")
                nc.scalar.activation(osb[:, 0:512], ps[0][:], AFT.Copy)
                nc.scalar.activation(osb[:, 512:1024], ps[1][:], AFT.Copy)
                r0 = base + i * 128
                nc.scalar.dma_start(out_d[r0:r0 + 128, :], osb[:])
        es.close()
    nc.finalize()
    return nc


def _prep_inputs(inputMatrix, dims, thresholds, lut):
    x = np.asarray(inputMatrix, dtype=np.float32)
    dims_l = [int(v) for v in np.asarray(dims).ravel()]
    thr = np.asarray(thresholds, dtype=np.float32).reshape(C, K - 1)
    lut = np.asarray(lut, dtype=np.float32)

    # thrcols [128, 15]: t0,t1,d21,t3,d43,t5,d65,t7,d87,t9,d109,t11,d1211,t13,d1413
    tcols = np.empty((C, 15), dtype=np.float32)
    tcols[:, 0] = thr[:, 0]
    pairs = [(1, 2), (3, 4), (5, 6), (7, 8), (9, 10), (11, 12), (13, 14)]
    for idx, (lo, hi) in enumerate(pairs):
        tcols[:, 1 + 2 * idx] = thr[:, lo]
        tcols[:, 2 + 2 * idx] = thr[:, hi] - thr[:, lo]
    thrcols = np.concatenate([tcols, tcols], axis=0)  # [128, 15]

    # lutT row tau*128 + 64e + c -> lut[m, c, 2*tau+e]
    lt = lut.reshape(M, C, 8, 2).transpose(2, 3, 1, 0).reshape(C * K, M)
    lutT = lt.astype(ml_dtypes.bfloat16)

    # ktab[p, tau] = 2*tau + p//64
    ktab = (2 * np.arange(8)[None, :] + (np.arange(128) // 64)[:, None]
            ).astype(np.float32)

    # xT per core: row 16*u + g = x_shard[g*256:(g+1)*256, u]
    xT = np.empty((N_CORES, N_CORE // G * D, G), dtype=np.float32)
    for i in range(N_CORES):
        xs = x[i * N_CORE:(i + 1) * N_CORE]
        xT[i] = xs.reshape(N_CORE // G, G, D).transpose(2, 0, 1).reshape(-1, G)

    # gather indices per stage: flat i = d*128 + 64*h + c
    #   row (in [_, HW_] view) = u * (N_CORE//HW_) + base//HW_ + h
    dims_a = np.asarray(dims_l, dtype=np.int64).reshape(C, DEPTH)
    gidx = np.empty((128, 32 * len(STAGES)), dtype=np.int16)

    def _rows(W, base, dlist):
        HW_ = W // 2
        vals = np.empty(128 * len(dlist), dtype=np.int16)
        for di, d in enumerate(dlist):
            for h in range(2):
                for c in range(C):
                    vals[di * 128 + 64 * h + c] = (
                        dims_a[c, d] * (N_CORE // HW_) + base // HW_ + h
                    )
        blk = vals.reshape(-1, 16).T  # [16, 8*len(dlist)]
        return np.tile(blk, (8, 1))

    for s, (W, base) in enumerate(zip(STAGES, _bases)):
        gidx[:, s * 32:(s + 1) * 32] = _rows(W, base, [0, 1, 2, 3])

    return xT, dims_l, thrcols, lutT, ktab, gidx


def _make_in_maps(xT, dims_l, thrcols, lutT, ktab, gidx):
    return [
        {
            "xT": np.ascontiguousarray(xT[i]),
            "thrcols": thrcols,
            "lutT": lutT,
            "ktab": ktab,
            "gidx": gidx,
        }
        for i in range(N_CORES)
    ]


def kernel(inputMatrix, dims, thresholds, lut, selection_matrix=None,
           tree_des_mat=None):
    from concourse.bass_utils import run_bass_kernel_spmd

    prep = _prep_inputs(inputMatrix, dims, thresholds, lut)
    nc = build_program(prep[1])
    in_maps = _make_in_maps(*prep)
    res = run_bass_kernel_spmd(nc, in_maps, list(range(N_CORES)))
    out = np.concatenate(
        [np.asarray(res.results[i]["out"]) for i in range(N_CORES)], axis=0
    )
    return out.astype(np.float32)


# revision 25
# speedup vs baseline: 1.0105x; 1.0105x over previous
import sys

if "/opt/trn_rl_repo" not in sys.path:
    sys.path.insert(0, "/opt/trn_rl_repo")

import numpy as np
import ml_dtypes

from concourse import bass, tile, bacc
from concourse.bass import mybir

F32 = mybir.dt.float32
F16 = mybir.dt.float16
BF16 = mybir.dt.bfloat16
I16 = mybir.dt.int16
I8 = mybir.dt.int8

N_CORES = 8
N_TOTAL = 32768
N_CORE = N_TOTAL // N_CORES  # 4096 rows per core
D = 1024
C = 64
K = 16
DEPTH = 4
M = 1024
G = 128                      # xT DRAM row granularity (floats)
STAGES = [512, 1024, 1024, 1024, 512]  # rows per stage (sum = N_CORE)
N_WARM_MM = 90                  # dummy matmuls to keep HAM warm during prologue
ALU = mybir.AluOpType
AFT = mybir.ActivationFunctionType

assert sum(STAGES) == N_CORE
_bases = np.cumsum([0] + STAGES[:-1]).tolist()
for _w, _b in zip(STAGES, _bases):
    assert _b % (_w // 2) == 0 and (_w // 2) % G == 0


def build_program(dims, repeat=1):
    """dims kept for signature compat; gather indices live in gidx input."""
    nc = bacc.Bacc()
    # xT row 16*u + g holds x[g*256:(g+1)*256, u] of this core's shard
    xT_d = nc.declare_dram_parameter("xT", [N_CORE // G * D, G], F32, isOutput=False)
    thr_d = nc.declare_dram_parameter("thrcols", [128, 15], F32, isOutput=False)
    lut_d = nc.declare_dram_parameter("lutT", [C * K, M], BF16, isOutput=False)
    ktab_d = nc.declare_dram_parameter("ktab", [128, 8], F32, isOutput=False)
    gidx_d = nc.declare_dram_parameter("gidx", [128, 32 * len(STAGES)], I16,
                                       isOutput=False)
    out_d = nc.declare_dram_parameter("out", [N_CORE, M], F16, isOutput=True)

    with tile.TileContext(nc) as tc:
        from contextlib import ExitStack
        es = ExitStack()
        pers = es.enter_context(tc.tile_pool(name="pers", bufs=1))

        def ptile(shape, dtype, name):
            return pers.tile(shape, dtype, name=name, tag=name)

        WMAX = max(STAGES)

        # ---- persistent tiles ----
        lutT = ptile([128, 8, M], BF16, "lutT_sb")       # [tau*128+64e+c, tau, m]
        thr = ptile([128, 15], F32, "thr_sb")
        ktab = ptile([128, 8], F32, "ktab_sb")
        gidx = ptile([128, 32 * len(STAGES)], I16, "gidx_sb")
        tmps = [ptile([128, WMAX // 2], F32, f"tmp{ti}_sb") for ti in range(7)]
        b0, b1, b2, sa, sb, sc, sd = tmps
        bi = ptile([128, WMAX], I16, "bi_sb")
        b0i = bi[:, :WMAX // 2]
        b1i = bi[:, WMAX // 2:]

        chpool = es.enter_context(tc.tile_pool(name="chpool", bufs=2))
        bkpool = es.enter_context(tc.tile_pool(name="bkpool", bufs=2))
        b2pool = es.enter_context(tc.tile_pool(name="b2pool", bufs=2))
        etpool = es.enter_context(tc.tile_pool(name="etpool", bufs=2))
        opool = es.enter_context(tc.tile_pool(name="opool", bufs=4))
        pspool = es.enter_context(
            tc.tile_pool(name="pspool", bufs=4, space=bass.MemorySpace.PSUM)
        )

        nc.sync.dma_start(gidx[:], gidx_d[:])
        nc.sync.dma_start(thr[:], thr_d[:])
        nc.sync.dma_start(ktab[:], ktab_d[:])
        for j in range(8):
            nc.sync.dma_start(lutT[:, j, :], lut_d[j * 128:(j + 1) * 128, :])

        # dummy matmuls keep the PE clock un-throttled through the prologue
        wps = pspool.tile([128, 512], F32, name="wps", tag="ps0")
        for _ in range(N_WARM_MM):
            nc.tensor.matmul(wps[:], lutT[:, 0, 0:128], lutT[:, 0, 0:512],
                             start=True, stop=True)

        def tcol(i):
            return thr[:, i:i + 1]

        from concourse.tile import add_dep_helper
        last_iseq = None
        stage_list = [sw for _ in range(repeat) for sw in zip(range(len(STAGES)),
                                                              STAGES, _bases)]
        for s, W, base in stage_list:
            HW_ = W // 2  # half-stage width
            # ---- gather: ch[p=(h,c), d, n'] = x[base+h*HW_+n', dims[4c+d]]
            ch = chpool.tile([128, DEPTH, HW_], F32, name="ch", tag="ch")
            b = HW_ // G
            src = xT_d[:] if b == 1 else xT_d[:].rearrange(
                "(a b) g -> a (b g)", b=b)
            g0 = s * 32
            nc.gpsimd.dma_gather(
                ch[:], src, gidx[:, g0:g0 + 32], 512, 512, HW_,
            )
            xd = [ch[:, d, :] for d in range(DEPTH)]

            def T(t):
                return t[:, :HW_]

            # ---- tree descent on [128=(h,c), HW_] ----
            i0 = nc.vector.tensor_scalar(T(b0), xd[0], tcol(0), None, ALU.is_gt)
            if last_iseq is not None:
                add_dep_helper(i0.ins, last_iseq.ins, sync=False,
                               reason="DVE order: ET compares before next descent")
            nc.vector.tensor_scalar(T(sa), T(b0), tcol(2), tcol(1), ALU.mult, ALU.add)
            nc.vector.tensor_copy(T(b0i), T(b0))
            nc.vector.tensor_tensor(T(b1), xd[1], T(sa), ALU.is_gt)

            nc.vector.tensor_scalar(T(sa), T(b1), tcol(4), tcol(3), ALU.mult, ALU.add)
            nc.vector.tensor_scalar(T(sb), T(b1), tcol(6), tcol(5), ALU.mult, ALU.add)
            nc.vector.tensor_copy(T(b1i), T(b1))
            nc.vector.copy_predicated(T(sa), T(b0i), T(sb))
            nc.vector.tensor_tensor(T(b2), xd[2], T(sa), ALU.is_gt)

            nc.vector.tensor_scalar(T(sa), T(b2), tcol(8), tcol(7), ALU.mult, ALU.add)
            nc.vector.tensor_scalar(T(sb), T(b2), tcol(10), tcol(9), ALU.mult, ALU.add)
            nc.vector.tensor_scalar(T(sc), T(b2), tcol(12), tcol(11), ALU.mult, ALU.add)
            nc.vector.tensor_scalar(T(sd), T(b2), tcol(14), tcol(13), ALU.mult, ALU.add)
            nc.vector.copy_predicated(T(sa), T(b1i), T(sb))
            nc.vector.copy_predicated(T(sc), T(b1i), T(sd))
            nc.vector.copy_predicated(T(sa), T(b0i), T(sc))
            nc.vector.tensor_tensor(T(sb), xd[3], T(sa), ALU.is_gt)  # b3 -> sb

            bk = bkpool.tile([128, HW_], BF16, name="bk", tag="bk")
            nc.vector.scalar_tensor_tensor(T(sc), T(b0), 2.0, T(b1), ALU.mult, ALU.add)
            nc.vector.scalar_tensor_tensor(T(sd), T(sc), 2.0, T(b2), ALU.mult, ALU.add)
            nc.vector.scalar_tensor_tensor(bk[:], T(sd), 2.0, T(sb), ALU.mult, ALU.add)

            # ---- duplicate bucket to both e-halves: b2k[64e+c, h*HW_+n'] ----
            b2k = b2pool.tile([128, W], BF16, name="b2k", tag="b2k")
            for e in range(2):
                for h in range(2):
                    _eng = nc.scalar if (s == 0 and h == 1) else nc.sync
                    _eng.dma_start(
                        b2k[64 * e:64 * e + 64, h * HW_:(h + 1) * HW_],
                        bk[64 * h:64 * h + 64, :],
                    )

            # ---- ET: et[p=(e,c), tau, nn] = (bucket == 2*tau + e) ----
            et = etpool.tile([128, 8, W], BF16, name="et", tag="et")
            for tau in range(8):
                last_iseq = nc.vector.tensor_scalar(
                    et[:, tau, :], b2k[:], ktab[:, tau:tau + 1], None, ALU.is_equal
                )

            # ---- matmul + output ----
            for i in range(W // 128):
                ps = [
                    pspool.tile([128, 512], F32, name=f"ps{mc}", tag=f"ps{mc}")
                    for mc in range(2)
                ]
                for tau in range(8):
                    lhsT = et[:, tau, i * 128:(i + 1) * 128]
                    for mc in range(2):
                        nc.tensor.matmul(
                            ps[mc][:], lhsT, lutT[:, tau, mc * 512:(mc + 1) * 512],
                            start=(tau == 0), stop=(tau == 7),
                        )
                osb = opool.tile([128, M], F16, name="osb", tag="osb")
                nc.scalar.activation(osb[:, 0:512], ps[0][:], AFT.Copy)
                nc.scalar.activation(osb[:, 512:1024], ps[1][:], AFT.Copy)
                r0 = base + i * 128
                nc.scalar.dma_start(out_d[r0:r0 + 128, :], osb[:])
        es.close()
    nc.finalize()
    return nc


def _prep_inputs(inputMatrix, dims, thresholds, lut):
    x = np.asarray(inputMatrix, dtype=np.float32)
    dims_l = [int(v) for v in np.asarray(dims).ravel()]
    thr = np.asarray(thresholds, dtype=np.float32).reshape(C, K - 1)
    lut = np.asarray(lut, dtype=np.float32)

    # thrcols [128, 15]: t0,t1,d21,t3,d43,t5,d65,t7,d87,t9,d109,t11,d1211,t13,d1413
    tcols = np.empty((C, 15), dtype=np.float32)
    tcols[:, 0] = thr[:, 0]
    pairs = [(1, 2), (3, 4), (5, 6), (7, 8), (9, 10), (11, 12), (13, 14)]
    for idx, (lo, hi) in enumerate(pairs):
        tcols[:, 1 + 2 * idx] = thr[:, lo]
        tcols[:, 2 + 2 * idx] = thr[:, hi] - thr[:, lo]
    thrcols = np.concatenate([tcols, tcols], axis=0)  # [128, 15]

    # lutT row tau*128 + 64e + c -> lut[m, c, 2*tau+e]
    lt = lut.reshape(M, C, 8, 2).transpose(2, 3, 1, 0).reshape(C * K, M)
    lutT = lt.astype(ml_dtypes.bfloat16)

    # ktab[p, tau] = 2*tau + p//64
    ktab = (2 * np.arange(8)[None, :] + (np.arange(128) // 64)[:, None]
            ).astype(np.float32)

    # xT per core: row 16*u + g = x_shard[g*256:(g+1)*256, u]
    xT = np.empty((N_CORES, N_CORE // G * D, G), dtype=np.float32)
    for i in range(N_CORES):
        xs = x[i * N_CORE:(i + 1) * N_CORE]
        xT[i] = xs.reshape(N_CORE // G, G, D).transpose(2, 0, 1).reshape(-1, G)

    # gather indices per stage: flat i = d*128 + 64*h + c
    #   row (in [_, HW_] view) = u * (N_CORE//HW_) + base//HW_ + h
    dims_a = np.asarray(dims_l, dtype=np.int64).reshape(C, DEPTH)
    gidx = np.empty((128, 32 * len(STAGES)), dtype=np.int16)

    def _rows(W, base, dlist):
        HW_ = W // 2
        vals = np.empty(128 * len(dlist), dtype=np.int16)
        for di, d in enumerate(dlist):
            for h in range(2):
                for c in range(C):
                    vals[di * 128 + 64 * h + c] = (
                        dims_a[c, d] * (N_CORE // HW_) + base // HW_ + h
                    )
        blk = vals.reshape(-1, 16).T  # [16, 8*len(dlist)]
        return np.tile(blk, (8, 1))

    for s, (W, base) in enumerate(zip(STAGES, _bases)):
        gidx[:, s * 32:(s + 1) * 32] = _rows(W, base, [0, 1, 2, 3])

    return xT, dims_l, thrcols, lutT, ktab, gidx


def _make_in_maps(xT, dims_l, thrcols, lutT, ktab, gidx):
    return [
        {
            "xT": np.ascontiguousarray(xT[i]),
            "thrcols": thrcols,
            "lutT": lutT,
            "ktab": ktab,
            "gidx": gidx,
        }
        for i in range(N_CORES)
    ]


def kernel(inputMatrix, dims, thresholds, lut, selection_matrix=None,
           tree_des_mat=None):
    from concourse.bass_utils import run_bass_kernel_spmd

    prep = _prep_inputs(inputMatrix, dims, thresholds, lut)
    nc = build_program(prep[1])
    in_maps = _make_in_maps(*prep)
    res = run_bass_kernel_spmd(nc, in_maps, list(range(N_CORES)))
    out = np.concatenate(
        [np.asarray(res.results[i]["out"]) for i in range(N_CORES)], axis=0
    )
    return out.astype(np.float32)


# revision 26
# speedup vs baseline: 1.0142x; 1.0038x over previous
import sys

if "/opt/trn_rl_repo" not in sys.path:
    sys.path.insert(0, "/opt/trn_rl_repo")

import numpy as np
import ml_dtypes

from concourse import bass, tile, bacc
from concourse.bass import mybir

F32 = mybir.dt.float32
F16 = mybir.dt.float16
BF16 = mybir.dt.bfloat16
I16 = mybir.dt.int16
I8 = mybir.dt.int8

N_CORES = 8
N_TOTAL = 32768
N_CORE = N_TOTAL // N_CORES  # 4096 rows per core
D = 1024
C = 64
K = 16
DEPTH = 4
M = 1024
G = 128                      # xT DRAM row granularity (floats)
STAGES = [512, 1024, 1024, 1024, 512]  # rows per stage (sum = N_CORE)
N_WARM_MM = 90                  # dummy matmuls to keep HAM warm during prologue
ALU = mybir.AluOpType
AFT = mybir.ActivationFunctionType

assert sum(STAGES) == N_CORE
_bases = np.cumsum([0] + STAGES[:-1]).tolist()
for _w, _b in zip(STAGES, _bases):
    assert _b % (_w // 2) == 0 and (_w // 2) % G == 0


def build_program(dims, repeat=1):
    """dims kept for signature compat; gather indices live in gidx input."""
    nc = bacc.Bacc()
    # xT row 16*u + g holds x[g*256:(g+1)*256, u] of this core's shard
    xT_d = nc.declare_dram_parameter("xT", [N_CORE // G * D, G], F32, isOutput=False)
    thr_d = nc.declare_dram_parameter("thrcols", [128, 15], F32, isOutput=False)
    lut_d = nc.declare_dram_parameter("lutT", [C * K, M], BF16, isOutput=False)
    ktab_d = nc.declare_dram_parameter("ktab", [128, 8], F32, isOutput=False)
    gidx_d = nc.declare_dram_parameter("gidx", [128, 32 * len(STAGES)], I16,
                                       isOutput=False)
    out_d = nc.declare_dram_parameter("out", [N_CORE, M], F16, isOutput=True)

    with tile.TileContext(nc) as tc:
        from contextlib import ExitStack
        es = ExitStack()
        pers = es.enter_context(tc.tile_pool(name="pers", bufs=1))

        def ptile(shape, dtype, name):
            return pers.tile(shape, dtype, name=name, tag=name)

        WMAX = max(STAGES)

        # ---- persistent tiles ----
        lutT = ptile([128, 8, M], BF16, "lutT_sb")       # [tau*128+64e+c, tau, m]
        thr = ptile([128, 15], F32, "thr_sb")
        ktab = ptile([128, 8], F32, "ktab_sb")
        gidx = ptile([128, 32 * len(STAGES)], I16, "gidx_sb")
        tmps = [ptile([128, WMAX // 2], F32, f"tmp{ti}_sb") for ti in range(7)]
        b0, b1, b2, sa, sb, sc, sd = tmps
        bi = ptile([128, WMAX], I16, "bi_sb")
        b0i = bi[:, :WMAX // 2]
        b1i = bi[:, WMAX // 2:]

        chpool = es.enter_context(tc.tile_pool(name="chpool", bufs=2))
        bkpool = es.enter_context(tc.tile_pool(name="bkpool", bufs=2))
        b2pool = es.enter_context(tc.tile_pool(name="b2pool", bufs=2))
        etpool = es.enter_context(tc.tile_pool(name="etpool", bufs=2))
        opool = es.enter_context(tc.tile_pool(name="opool", bufs=4))
        pspool = es.enter_context(
            tc.tile_pool(name="pspool", bufs=4, space=bass.MemorySpace.PSUM)
        )

        nc.sync.dma_start(gidx[:], gidx_d[:])
        nc.sync.dma_start(thr[:], thr_d[:])
        nc.sync.dma_start(ktab[:], ktab_d[:])
        for j in range(8):
            nc.sync.dma_start(lutT[:, j, :], lut_d[j * 128:(j + 1) * 128, :])

        # dummy matmuls keep the PE clock un-throttled through the prologue
        wps = pspool.tile([128, 512], F32, name="wps", tag="ps0")
        for _ in range(N_WARM_MM):
            nc.tensor.matmul(wps[:], lutT[:, 0, 0:128], lutT[:, 0, 0:512],
                             start=True, stop=True)

        def tcol(i):
            return thr[:, i:i + 1]

        from concourse.tile import add_dep_helper
        last_iseq = None
        stage_list = [sw for _ in range(repeat) for sw in zip(range(len(STAGES)),
                                                              STAGES, _bases)]
        for s, W, base in stage_list:
            HW_ = W // 2  # half-stage width
            # ---- gather: ch[p=(h,c), d, n'] = x[base+h*HW_+n', dims[4c+d]]
            ch = chpool.tile([128, DEPTH, HW_], F32, name="ch", tag="ch")
            b = HW_ // G
            src = xT_d[:] if b == 1 else xT_d[:].rearrange(
                "(a b) g -> a (b g)", b=b)
            g0 = s * 32
            nc.gpsimd.dma_gather(
                ch[:], src, gidx[:, g0:g0 + 32], 512, 512, HW_,
            )
            xd = [ch[:, d, :] for d in range(DEPTH)]

            def T(t):
                return t[:, :HW_]

            # ---- tree descent on [128=(h,c), HW_] ----
            i0 = nc.vector.tensor_scalar(T(b0), xd[0], tcol(0), None, ALU.is_gt)
            if last_iseq is not None:
                add_dep_helper(i0.ins, last_iseq.ins, sync=False,
                               reason="DVE order: ET compares before next descent")
            nc.vector.tensor_scalar(T(sa), T(b0), tcol(2), tcol(1), ALU.mult, ALU.add)
            nc.vector.tensor_copy(T(b0i), T(b0))
            nc.vector.tensor_tensor(T(b1), xd[1], T(sa), ALU.is_gt)

            nc.vector.tensor_scalar(T(sa), T(b1), tcol(4), tcol(3), ALU.mult, ALU.add)
            nc.vector.tensor_scalar(T(sb), T(b1), tcol(6), tcol(5), ALU.mult, ALU.add)
            nc.vector.tensor_copy(T(b1i), T(b1))
            nc.vector.copy_predicated(T(sa), T(b0i), T(sb))
            nc.vector.tensor_tensor(T(b2), xd[2], T(sa), ALU.is_gt)

            nc.vector.tensor_scalar(T(sa), T(b2), tcol(8), tcol(7), ALU.mult, ALU.add)
            nc.vector.tensor_scalar(T(sb), T(b2), tcol(10), tcol(9), ALU.mult, ALU.add)
            nc.vector.tensor_scalar(T(sc), T(b2), tcol(12), tcol(11), ALU.mult, ALU.add)
            nc.vector.tensor_scalar(T(sd), T(b2), tcol(14), tcol(13), ALU.mult, ALU.add)
            nc.vector.copy_predicated(T(sa), T(b1i), T(sb))
            nc.vector.copy_predicated(T(sc), T(b1i), T(sd))
            nc.vector.copy_predicated(T(sa), T(b0i), T(sc))
            nc.vector.tensor_tensor(T(sb), xd[3], T(sa), ALU.is_gt)  # b3 -> sb

            bk = bkpool.tile([128, HW_], BF16, name="bk", tag="bk")
            nc.vector.scalar_tensor_tensor(T(sc), T(b0), 2.0, T(b1), ALU.mult, ALU.add)
            nc.vector.scalar_tensor_tensor(T(sd), T(sc), 2.0, T(b2), ALU.mult, ALU.add)
            nc.vector.scalar_tensor_tensor(bk[:], T(sd), 2.0, T(sb), ALU.mult, ALU.add)

            # ---- duplicate bucket to both e-halves: b2k[64e+c, h*HW_+n'] ----
            b2k = b2pool.tile([128, W], BF16, name="b2k", tag="b2k")
            for e in range(2):
                for h in range(2):
                    _eng = nc.scalar if (s == 0 and h == 1) else nc.sync
                    _eng.dma_start(
                        b2k[64 * e:64 * e + 64, h * HW_:(h + 1) * HW_],
                        bk[64 * h:64 * h + 64, :],
                    )

            # ---- ET: et[p=(e,c), tau, nn] = (bucket == 2*tau + e) ----
            et = etpool.tile([128, 8, W], BF16, name="et", tag="et")
            for hh in range(2):
                sl = slice(hh * HW_, (hh + 1) * HW_)
                for tau in range(8):
                    last_iseq = nc.vector.tensor_scalar(
                        et[:, tau, sl], b2k[:, sl], ktab[:, tau:tau + 1], None,
                        ALU.is_equal
                    )

            # ---- matmul + output ----
            for i in range(W // 128):
                ps = [
                    pspool.tile([128, 512], F32, name=f"ps{mc}", tag=f"ps{mc}")
                    for mc in range(2)
                ]
                for tau in range(8):
                    lhsT = et[:, tau, i * 128:(i + 1) * 128]
                    for mc in range(2):
                        nc.tensor.matmul(
                            ps[mc][:], lhsT, lutT[:, tau, mc * 512:(mc + 1) * 512],
                            start=(tau == 0), stop=(tau == 7),
                        )
                osb = opool.tile([128, M], F16, name="osb", tag="osb")
                nc.scalar.activation(osb[:, 0:512], ps[0][:], AFT.Copy)
                nc.scalar.activation(osb[:, 512:1024], ps[1][:], AFT.Copy)
                r0 = base + i * 128
                nc.scalar.dma_start(out_d[r0:r0 + 128, :], osb[:])
        es.close()
    nc.finalize()
    return nc


def _prep_inputs(inputMatrix, dims, thresholds, lut):
    x = np.asarray(inputMatrix, dtype=np.float32)
    dims_l = [int(v) for v in np.asarray(dims).ravel()]
    thr = np.asarray(thresholds, dtype=np.float32).reshape(C, K - 1)
    lut = np.asarray(lut, dtype=np.float32)

    # thrcols [128, 15]: t0,t1,d21,t3,d43,t5,d65,t7,d87,t9,d109,t11,d1211,t13,d1413
    tcols = np.empty((C, 15), dtype=np.float32)
    tcols[:, 0] = thr[:, 0]
    pairs = [(1, 2), (3, 4), (5, 6), (7, 8), (9, 10), (11, 12), (13, 14)]
    for idx, (lo, hi) in enumerate(pairs):
        tcols[:, 1 + 2 * idx] = thr[:, lo]
        tcols[:, 2 + 2 * idx] = thr[:, hi] - thr[:, lo]
    thrcols = np.concatenate([tcols, tcols], axis=0)  # [128, 15]

    # lutT row tau*128 + 64e + c -> lut[m, c, 2*tau+e]
    lt = lut.reshape(M, C, 8, 2).transpose(2, 3, 1, 0).reshape(C * K, M)
    lutT = lt.astype(ml_dtypes.bfloat16)

    # ktab[p, tau] = 2*tau + p//64
    ktab = (2 * np.arange(8)[None, :] + (np.arange(128) // 64)[:, None]
            ).astype(np.float32)

    # xT per core: row 16*u + g = x_shard[g*256:(g+1)*256, u]
    xT = np.empty((N_CORES, N_CORE // G * D, G), dtype=np.float32)
    for i in range(N_CORES):
        xs = x[i * N_CORE:(i + 1) * N_CORE]
        xT[i] = xs.reshape(N_CORE // G, G, D).transpose(2, 0, 1).reshape(-1, G)

    # gather indices per stage: flat i = d*128 + 64*h + c
    #   row (in [_, HW_] view) = u * (N_CORE//HW_) + base//HW_ + h
    dims_a = np.asarray(dims_l, dtype=np.int64).reshape(C, DEPTH)
    gidx = np.empty((128, 32 * len(STAGES)), dtype=np.int16)

    def _rows(W, base, dlist):
        HW_ = W // 2
        vals = np.empty(128 * len(dlist), dtype=np.int16)
        for di, d in enumerate(dlist):
            for h in range(2):
                for c in range(C):
                    vals[di * 128 + 64 * h + c] = (
                        dims_a[c, d] * (N_CORE // HW_) + base // HW_ + h
                    )
        blk = vals.reshape(-1, 16).T  # [16, 8*len(dlist)]
        return np.tile(blk, (8, 1))

    for s, (W, base) in enumerate(zip(STAGES, _bases)):
        gidx[:, s * 32:(s + 1) * 32] = _rows(W, base, [0, 1, 2, 3])

    return xT, dims_l, thrcols, lutT, ktab, gidx


def _make_in_maps(xT, dims_l, thrcols, lutT, ktab, gidx):
    return [
        {
            "xT": np.ascontiguousarray(xT[i]),
            "thrcols": thrcols,
            "lutT": lutT,
            "ktab": ktab,
            "gidx": gidx,
        }
        for i in range(N_CORES)
    ]


def kernel(inputMatrix, dims, thresholds, lut, selection_matrix=None,
           tree_des_mat=None):
    from concourse.bass_utils import run_bass_kernel_spmd

    prep = _prep_inputs(inputMatrix, dims, thresholds, lut)
    nc = build_program(prep[1])
    in_maps = _make_in_maps(*prep)
    res = run_bass_kernel_spmd(nc, in_maps, list(range(N_CORES)))
    out = np.concatenate(
        [np.asarray(res.results[i]["out"]) for i in range(N_CORES)], axis=0
    )
    return out.astype(np.float32)
